# revision 5
# baseline (speedup 1.0000x reference)
"""Pairwise Euclidean distance matrix on 8 TRN2 NeuronCores (Bass/Tile).

out[i, j] = ||x[j] - x[i]||_2 for x [4096, 512] fp32.

Distance symmetry: out = out.T, so only ~half the blocks are computed.
Half-ring decomposition: core c owns query block c and computes it
against key blocks {c, c+1, .., c+4 mod 8} — 5 of 8 blocks, perfectly
balanced and SPMD-uniform. Ring distance 1..3 blocks are mirrored into
their transposed position on the host; distance 0/4 covered directly.

Layout: queries on PSUM partitions, keys on the free axis. The Gram
part runs as fp8 e4m3 DoubleRow matmuls (2 fp8 weights/cell, 2 MACs/
cycle): query subblocks [128c,2,128q] stationary, key chunks
[128c,2,512k] moving; -2 is pre-folded into the quantized queries. A
tiny [2,128]x[2,512] bf16 augmentation matmul per PSUM tile adds
sq_m (per-query) + sq_n (per-key) into the same accumulation, so PSUM
holds d^2 directly. Epilogue per 4-bank group is then a single big ACT
Sqrt (PSUM -> SBUF bf16) and one output DMA in SBUF-native layout
(host unscrambles). No DVE work at all; quantization error lands
~7e-3 on the harness metric (gate 2e-2). Diagonal d^2 can go slightly
negative under fp8 -> NaN after sqrt; host overwrites the diagonal.
"""

import numpy as np
import ml_dtypes

import concourse.bass as bass
import concourse.bacc as bacc
import concourse.tile as tile
from concourse.bass_utils import run_bass_kernel_spmd

mybir = bass.mybir

N = 4096          # number of points
D = 512           # feature dim
NCORES = 8
QB = N // NCORES  # 512 queries per core
RB = 5            # row blocks per core (half-ring)
KEYS = RB * QB    # 2560 keys per core
NC = 5            # key chunks of 512 per core
NS = 4            # query subblocks of 128

_FP8 = mybir.dt.float8e4
_BF16 = mybir.dt.bfloat16
_F32 = mybir.dt.float32

_nc_cache = {}


def _build():
    if "nc" in _nc_cache:
        return _nc_cache["nc"]
    nc = bacc.Bacc("TRN2", target_bir_lowering=False, debug=False)

    # keys: [128, chunk, k-subtile, 512] e4m3 packed host-side so each
    # chunk DMA reads 2048B contiguous per partition
    xp = nc.dram_tensor("xp", [128, NC * 4 * 512], _FP8, kind="ExternalInput")
    # queries: [128, k-subtile, 512] e4m3, pre-scaled by -2
    q = nc.dram_tensor("q", [128, 4 * QB], _FP8, kind="ExternalInput")
    # augmentation rows (bf16): cols 0:QB = (sq_m; ones),
    # cols QB:QB+KEYS = (ones; sq_n)
    aug = nc.dram_tensor("aug", [2, QB + KEYS], _BF16, kind="ExternalInput")
    # output in SBUF-native layout: col block t=c*4+s holds
    # d[query s*128+p, key chunk c]
    out = nc.dram_tensor("out", [128, NC * NS * 512], _BF16, kind="ExternalOutput")

    sqrt = mybir.ActivationFunctionType.Sqrt
    dr = mybir.MatmulPerfMode.DoubleRow

    with tile.TileContext(nc) as tc:
        with (
            tc.tile_pool(name="xd", bufs=1) as xd,
            tc.tile_pool(name="op", bufs=3) as op,
            tc.tile_pool(name="ps", bufs=2, space="PSUM") as pp,
        ):
            # ACT sqrt table preload (~2.7us) rides the startup phase
            dumm = xd.tile([128, 1], _F32, tag="dumm", name="dumm")
            nc.vector.memset(dumm[:], 1.0)
            nc.scalar.activation(dumm[:], dumm[:], sqrt, bias=0.0, scale=1.0)

            # aug rows early on gpsimd (its first load)
            t_aug = xd.tile([2, QB + KEYS], _BF16, tag="aug", name="aug")
            nc.gpsimd.dma_start(t_aug[:], aug.ap())

            # key chunks + queries: flat 2048B-contiguous loads (a 4D
            # dst AP would emit 512B descriptors, ~75GB/s); the matmul
            # slices come from a strided AP view instead.
            t_k, kv = [], []
            for c in range(NC):
                t = xd.tile([128, 4 * 512], _FP8, tag=f"k{c}", name=f"k{c}")
                nc.sync.dma_start(t[:], xp.ap()[:, c * 2048 : (c + 1) * 2048])
                t_k.append(t)
                kv.append(t[:].rearrange("p (kp k n) -> p kp k n", kp=2, k=2))
                if c == 0:
                    t_q = xd.tile([128, 4 * QB], _FP8, tag="q", name="q")
                    nc.sync.dma_start(t_q[:], q.ap())
                    qv = t_q[:].rearrange("p (kp k n) -> p kp k n", kp=2, k=2)

            # PE warmup: HAM clock gate is cold (1.2 GHz) until ~3.4us of
            # sustained activity; bridge the wait for the first key chunk.
            warm = xd.tile([128, QB], _BF16, tag="warm", name="warm")
            nc.vector.memset(warm[:], 0.0)
            wps = pp.tile([128, NS * 512], _F32, tag="ps", name="wps")
            for _ in range(3):
                nc.tensor.matmul(
                    wps[:, 0:QB], warm[:, 0:128], warm[:], start=True, stop=True
                )

            for c in range(NC):
                psg = pp.tile([128, NS * 512], _F32, tag="ps", name=f"ps{c}")
                o = op.tile([128, NS * 512], _BF16, tag="o", name=f"o{c}")
                # s-major so the first supertile half completes early;
                # ACT+store run per half, halving the PSUM-recycle and
                # end-of-kernel latency.
                for s in range(NS):
                    sl = slice(s * 512, (s + 1) * 512)
                    for kp in (0, 1):
                        nc.tensor.matmul(
                            psg[:, sl],
                            qv[:, kp, :, s * 128 : (s + 1) * 128],
                            kv[c][:, kp, :, :],
                            start=(kp == 0),
                            stop=False,
                            perf_mode=dr,
                        )
                    nc.tensor.matmul(
                        psg[:, sl],
                        t_aug[:, s * 128 : (s + 1) * 128],
                        t_aug[:, QB + c * 512 : QB + (c + 1) * 512],
                        start=False,
                        stop=True,
                    )
                    if s % 2 == 1:
                        hl = slice((s - 1) * 512, (s + 1) * 512)
                        nc.scalar.activation(
                            o[:, hl], psg[:, hl], sqrt, bias=0.0, scale=1.0
                        )
                        dst = out.ap()[
                            :, c * 2048 + (s - 1) * 512 : c * 2048 + (s + 1) * 512
                        ]
                        eng = nc.gpsimd if s == 1 else nc.sync
                        eng.dma_start(dst, o[:, hl])

    nc.compile()
    _nc_cache["nc"] = nc
    return nc


def _ring(c):
    return [(c + t) % NCORES for t in range(RB)]


def _prep_inputs(x: np.ndarray):
    x = np.ascontiguousarray(x, dtype=np.float32)
    x8 = x.astype(ml_dtypes.float8_e4m3)       # keys [N, D]
    q8 = (-2.0 * x).astype(ml_dtypes.float8_e4m3)
    sqv = np.einsum("nd,nd->n", x.astype(np.float64), x.astype(np.float64))
    sqb = sqv.astype(ml_dtypes.bfloat16)
    ones = np.ones(N, dtype=ml_dtypes.bfloat16)

    in_maps = []
    for c in range(NCORES):
        r0 = c * QB
        keycols = np.concatenate(
            [np.arange(r * QB, (r + 1) * QB) for r in _ring(c)]
        )
        # keys: [p, chunk, ksub, n] with feature 128*ksub+p of key keycols[.]
        kc = x8[keycols, :].reshape(NC, 512, 4, 128)  # [c, n, k, p]
        xp_pack = kc.transpose(3, 0, 2, 1).reshape(128, NC * 4 * 512)
        # queries: [p, ksub, j]
        qc = q8[r0 : r0 + QB, :].reshape(QB, 4, 128)
        q_pack = qc.transpose(2, 1, 0).reshape(128, 4 * QB)
        aug_pack = np.empty((2, QB + KEYS), dtype=ml_dtypes.bfloat16)
        aug_pack[0, 0:QB] = sqb[r0 : r0 + QB]
        aug_pack[1, 0:QB] = ones[0:QB]
        aug_pack[0, QB:] = ones[0:KEYS]
        aug_pack[1, QB:] = sqb[keycols]
        in_maps.append(
            {
                "xp": np.ascontiguousarray(xp_pack),
                "q": np.ascontiguousarray(q_pack),
                "aug": np.ascontiguousarray(aug_pack),
            }
        )
    return in_maps


def run(x: np.ndarray, trace: bool = False, tmpdir: str | None = None):
    nc = _build()
    in_maps = _prep_inputs(x)
    res = run_bass_kernel_spmd(
        nc, in_maps, list(range(NCORES)), trace=trace, tmpdir=tmpdir
    )
    full = np.empty((N, N), dtype=np.float32)
    for c in range(NCORES):
        o = res.results[c]["out"].astype(np.float32)
        # [p, c, s, n] -> blk[q = s*128+p, key = c*512+n]
        blk = o.reshape(128, NC, NS, 512).transpose(2, 0, 1, 3).reshape(QB, KEYS)
        for t, r in enumerate(_ring(c)):
            b = blk[:, t * QB : (t + 1) * QB]  # [queries blk c, keys blk r]
            full[r * QB : (r + 1) * QB, c * QB : (c + 1) * QB] = b.T
            if t in (1, 2, 3):  # ring distance 1..3: mirror
                full[c * QB : (c + 1) * QB, r * QB : (r + 1) * QB] = b
    np.fill_diagonal(full, 0.0)
    return full, res


def kernel(x: np.ndarray) -> np.ndarray:
    out, _ = run(x, trace=False)
    return out


# revision 7
# speedup vs baseline: 1.0725x; 1.0725x over previous
"""Pairwise Euclidean distance matrix on 8 TRN2 NeuronCores (Bass/Tile).

out[i, j] = ||x[j] - x[i]||_2 for x [4096, 512] fp32.

Distance symmetry: out = out.T, so only ~half the blocks are computed.
Half-ring decomposition: core c owns query block c and computes it
against key blocks {c, c+1, .., c+4 mod 8} — 5 of 8 blocks, perfectly
balanced and SPMD-uniform. Ring distance 1..3 blocks are mirrored into
their transposed position on the host; distance 0/4 covered directly.

Layout: queries on PSUM partitions, keys on the free axis. The Gram
part runs as fp8 e4m3 DoubleRow matmuls (2 fp8 weights/cell, 2 MACs/
cycle): query subblocks [128c,2,128q] stationary, key chunks
[128c,2,512k] moving; -2 is pre-folded into the quantized queries. A
tiny [2,128]x[2,512] bf16 augmentation matmul per PSUM tile adds
sq_m (per-query) + sq_n (per-key) into the same accumulation, so PSUM
holds d^2 directly. Epilogue per 4-bank group is then a single big ACT
Sqrt (PSUM -> SBUF bf16) and one output DMA in SBUF-native layout
(host unscrambles). No DVE work at all; quantization error lands
~7e-3 on the harness metric (gate 2e-2). Diagonal d^2 can go slightly
negative under fp8 -> NaN after sqrt; host overwrites the diagonal.
"""

import numpy as np
import ml_dtypes

import concourse.bass as bass
import concourse.bacc as bacc
import concourse.tile as tile
from concourse.bass_utils import run_bass_kernel_spmd

mybir = bass.mybir

N = 4096          # number of points
D = 512           # feature dim
NCORES = 8
QB = N // NCORES  # 512 queries per core
RB = 5            # row blocks per core (half-ring)
KEYS = RB * QB    # 2560 keys per core
NC = 5            # key chunks of 512 per core
NS = 4            # query subblocks of 128

_FP8 = mybir.dt.float8e4
_BF16 = mybir.dt.bfloat16
_F32 = mybir.dt.float32

_nc_cache = {}


def _build():
    if "nc" in _nc_cache:
        return _nc_cache["nc"]
    nc = bacc.Bacc("TRN2", target_bir_lowering=False, debug=False)

    # keys: [128, chunk, k-subtile, 512] e4m3 packed host-side so each
    # chunk DMA reads 2048B contiguous per partition
    xp = nc.dram_tensor("xp", [128, NC * 4 * 512], _FP8, kind="ExternalInput")
    # queries: [128, k-subtile, 512] e4m3, pre-scaled by -2
    q = nc.dram_tensor("q", [128, 4 * QB], _FP8, kind="ExternalInput")
    # augmentation rows (bf16): cols 0:QB = (sq_m; ones),
    # cols QB:QB+KEYS = (ones; sq_n)
    aug = nc.dram_tensor("aug", [2, QB + KEYS], _BF16, kind="ExternalInput")
    # output in SBUF-native layout: col block t=c*4+s holds
    # d[query s*128+p, key chunk c]
    out = nc.dram_tensor("out", [128, NC * NS * 512], _BF16, kind="ExternalOutput")

    sqrt = mybir.ActivationFunctionType.Sqrt
    dr = mybir.MatmulPerfMode.DoubleRow

    with tile.TileContext(nc) as tc:
        with (
            tc.tile_pool(name="xd", bufs=1) as xd,
            tc.tile_pool(name="op", bufs=3) as op,
            tc.tile_pool(name="ps", bufs=2, space="PSUM") as pp,
        ):
            # Inputs spread over four engines' DMA rings (same-engine
            # DMAs serialize on that ring, and each DMA has ~2.5us
            # trigger+completion latency): first-needed tensors get
            # their own ring. Flat 2048B-contiguous loads; matmul
            # slices come from strided AP views.
            t_k, kv = [], []
            for c in range(NC):
                t = xd.tile([128, 4 * 512], _FP8, tag=f"k{c}", name=f"k{c}")
                eng = [nc.sync, nc.scalar, nc.gpsimd, nc.sync, nc.scalar][c]
                eng.dma_start(t[:], xp.ap()[:, c * 2048 : (c + 1) * 2048])
                t_k.append(t)
                kv.append(t[:].rearrange("p (kp k n) -> p kp k n", kp=2, k=2))
                if c == 0:
                    t_q = xd.tile([128, 4 * QB], _FP8, tag="q", name="q")
                    nc.gpsimd.dma_start(t_q[:], q.ap())
                    qv = t_q[:].rearrange("p (kp k n) -> p kp k n", kp=2, k=2)
                    t_aug = xd.tile(
                        [2, QB + KEYS], _BF16, tag="aug", name="aug"
                    )
                    nc.gpsimd.dma_start(t_aug[:], aug.ap())

            # PE warmup: HAM clock gate is cold (1.2 GHz) until ~3.4us of
            # sustained activity; bridge the wait for the first key chunk.
            warm = xd.tile([128, QB], _BF16, tag="warm", name="warm")
            nc.vector.memset(warm[:], 0.0)
            # ACT sqrt table preload (~2.7us) rides the startup phase
            nc.scalar.activation(
                warm[:, 0:1], warm[:, 0:1], sqrt, bias=0.0, scale=1.0
            )
            wps = pp.tile([128, NS * 512], _F32, tag="ps", name="wps")
            for _ in range(4):
                nc.tensor.matmul(
                    wps[:, 0:QB], warm[:, 0:128], warm[:], start=True, stop=True
                )

            for c in range(NC):
                psg = pp.tile([128, NS * 512], _F32, tag="ps", name=f"ps{c}")
                o = op.tile([128, NS * 512], _BF16, tag="o", name=f"o{c}")
                # kp-major: consecutive matmuls hit different PSUM banks
                # so fills pipeline (same-bank back-to-back accumulation
                # exposes the ~250-cycle drain). Augs pair s0,s1 then
                # s2,s3 so ACT+store can run per supertile half.
                for kp in (0, 1):
                    for s in range(NS):
                        nc.tensor.matmul(
                            psg[:, s * 512 : (s + 1) * 512],
                            qv[:, kp, :, s * 128 : (s + 1) * 128],
                            kv[c][:, kp, :, :],
                            start=(kp == 0),
                            stop=False,
                            perf_mode=dr,
                        )
                for h in (0, 1):
                    for s in (2 * h, 2 * h + 1):
                        nc.tensor.matmul(
                            psg[:, s * 512 : (s + 1) * 512],
                            t_aug[:, s * 128 : (s + 1) * 128],
                            t_aug[:, QB + c * 512 : QB + (c + 1) * 512],
                            start=False,
                            stop=True,
                        )
                    hl = slice(h * 1024, (h + 1) * 1024)
                    nc.scalar.activation(
                        o[:, hl], psg[:, hl], sqrt, bias=0.0, scale=1.0
                    )
                    dst = out.ap()[
                        :, c * 2048 + h * 1024 : c * 2048 + (h + 1) * 1024
                    ]
                    eng = nc.gpsimd if h == 0 else nc.sync
                    eng.dma_start(dst, o[:, hl])

    nc.compile()
    _nc_cache["nc"] = nc
    return nc


def _ring(c):
    return [(c + t) % NCORES for t in range(RB)]


def _prep_inputs(x: np.ndarray):
    x = np.ascontiguousarray(x, dtype=np.float32)
    x8 = x.astype(ml_dtypes.float8_e4m3)       # keys [N, D]
    q8 = (-2.0 * x).astype(ml_dtypes.float8_e4m3)
    sqv = np.einsum("nd,nd->n", x.astype(np.float64), x.astype(np.float64))
    sqb = sqv.astype(ml_dtypes.bfloat16)
    ones = np.ones(N, dtype=ml_dtypes.bfloat16)

    in_maps = []
    for c in range(NCORES):
        r0 = c * QB
        keycols = np.concatenate(
            [np.arange(r * QB, (r + 1) * QB) for r in _ring(c)]
        )
        # keys: [p, chunk, ksub, n] with feature 128*ksub+p of key keycols[.]
        kc = x8[keycols, :].reshape(NC, 512, 4, 128)  # [c, n, k, p]
        xp_pack = kc.transpose(3, 0, 2, 1).reshape(128, NC * 4 * 512)
        # queries: [p, ksub, j]
        qc = q8[r0 : r0 + QB, :].reshape(QB, 4, 128)
        q_pack = qc.transpose(2, 1, 0).reshape(128, 4 * QB)
        aug_pack = np.empty((2, QB + KEYS), dtype=ml_dtypes.bfloat16)
        aug_pack[0, 0:QB] = sqb[r0 : r0 + QB]
        aug_pack[1, 0:QB] = ones[0:QB]
        aug_pack[0, QB:] = ones[0:KEYS]
        aug_pack[1, QB:] = sqb[keycols]
        in_maps.append(
            {
                "xp": np.ascontiguousarray(xp_pack),
                "q": np.ascontiguousarray(q_pack),
                "aug": np.ascontiguousarray(aug_pack),
            }
        )
    return in_maps


def run(x: np.ndarray, trace: bool = False, tmpdir: str | None = None):
    nc = _build()
    in_maps = _prep_inputs(x)
    res = run_bass_kernel_spmd(
        nc, in_maps, list(range(NCORES)), trace=trace, tmpdir=tmpdir
    )
    full = np.empty((N, N), dtype=np.float32)
    for c in range(NCORES):
        o = res.results[c]["out"].astype(np.float32)
        # [p, c, s, n] -> blk[q = s*128+p, key = c*512+n]
        blk = o.reshape(128, NC, NS, 512).transpose(2, 0, 1, 3).reshape(QB, KEYS)
        for t, r in enumerate(_ring(c)):
            b = blk[:, t * QB : (t + 1) * QB]  # [queries blk c, keys blk r]
            full[r * QB : (r + 1) * QB, c * QB : (c + 1) * QB] = b.T
            if t in (1, 2, 3):  # ring distance 1..3: mirror
                full[c * QB : (c + 1) * QB, r * QB : (r + 1) * QB] = b
    np.fill_diagonal(full, 0.0)
    return full, res


def kernel(x: np.ndarray) -> np.ndarray:
    out, _ = run(x, trace=False)
    return out


# revision 9
# speedup vs baseline: 1.4614x; 1.3626x over previous
"""Pairwise Euclidean distance matrix on 8 TRN2 NeuronCores (Bass/Tile).

out[i, j] = ||x[j] - x[i]||_2 for x [4096, 512] fp32.

Distance symmetry: out = out.T, so only ~half the blocks are computed.
Half-ring decomposition: core c owns query block c and computes it
against key blocks {c, c+1, .., c+4 mod 8} — 5 of 8 blocks, perfectly
balanced and SPMD-uniform. Ring distance 1..3 blocks are mirrored into
their transposed position on the host; distance 0/4 covered directly.

Layout: queries on PSUM partitions, keys on the free axis. The Gram
part runs as fp8 e4m3 DoubleRow matmuls (2 fp8 weights/cell, 2 MACs/
cycle): query subblocks [128c,2,128q] stationary, key chunks
[128c,2,512k] moving; -2 is pre-folded into the quantized queries. A
tiny [2,128]x[2,512] bf16 augmentation matmul per PSUM tile adds
sq_m (per-query) + sq_n (per-key) into the same accumulation, so PSUM
holds d^2 directly. Epilogue per 4-bank group is then a single big ACT
Sqrt (PSUM -> SBUF bf16) and one output DMA in SBUF-native layout
(host unscrambles). No DVE work at all; quantization error lands
~7e-3 on the harness metric (gate 2e-2). Diagonal d^2 can go slightly
negative under fp8 -> NaN after sqrt; host overwrites the diagonal.
"""

import numpy as np
import ml_dtypes

import concourse.bass as bass
import concourse.bacc as bacc
import concourse.tile as tile
from concourse.bass_utils import run_bass_kernel_spmd

mybir = bass.mybir

N = 4096          # number of points
D = 512           # feature dim
NCORES = 8
QB = N // NCORES  # 512 queries per core
RB = 5            # row blocks per core (half-ring)
KEYS = RB * QB    # 2560 keys per core
NC = 5            # key chunks of 512 per core
NS = 4            # query subblocks of 128

_FP8 = mybir.dt.float8e4
_BF16 = mybir.dt.bfloat16
_F32 = mybir.dt.float32

_nc_cache = {}


def _build():
    if "nc" in _nc_cache:
        return _nc_cache["nc"]
    nc = bacc.Bacc("TRN2", target_bir_lowering=False, debug=False)

    # keys: [128, chunk, k-subtile, 512] e4m3 packed host-side so each
    # chunk DMA reads 2048B contiguous per partition
    xp = nc.dram_tensor("xp", [128, NC * 4 * 512], _FP8, kind="ExternalInput")
    # queries: [128, k-subtile, 512] e4m3, pre-scaled by -2
    q = nc.dram_tensor("q", [128, 4 * QB], _FP8, kind="ExternalInput")
    # augmentation rows (bf16): cols 0:QB = (sq_m; ones),
    # cols QB:QB+KEYS = (ones; sq_n)
    aug = nc.dram_tensor("aug", [2, QB + KEYS], _BF16, kind="ExternalInput")
    # output in SBUF-native layout: col block t=c*4+s holds
    # d[query s*128+p, key chunk c]
    out = nc.dram_tensor("out", [128, NC * NS * 512], _BF16, kind="ExternalOutput")

    sqrt = mybir.ActivationFunctionType.Sqrt
    dr = mybir.MatmulPerfMode.DoubleRow

    with tile.TileContext(nc) as tc:
        with (
            tc.tile_pool(name="xd", bufs=1) as xd,
            tc.tile_pool(name="op", bufs=3) as op,
            tc.tile_pool(name="ps", bufs=2, space="PSUM") as pp,
        ):
            # Inputs spread over four engines' DMA rings (same-engine
            # DMAs serialize on that ring, and each DMA has ~2.5us
            # trigger+completion latency): first-needed tensors get
            # their own ring. Flat 2048B-contiguous loads; matmul
            # slices come from strided AP views.
            t_k, kv = [], []
            for c in range(NC):
                t = xd.tile([128, 4 * 512], _FP8, tag=f"k{c}", name=f"k{c}")
                eng = [nc.sync, nc.scalar, nc.gpsimd, nc.sync, nc.scalar][c]
                eng.dma_start(t[:], xp.ap()[:, c * 2048 : (c + 1) * 2048])
                t_k.append(t)
                kv.append(t[:].rearrange("p (kp k n) -> p kp k n", kp=2, k=2))
                if c == 0:
                    t_q = xd.tile([128, 4 * QB], _FP8, tag="q", name="q")
                    nc.gpsimd.dma_start(t_q[:], q.ap())
                    qv = t_q[:].rearrange("p (kp k n) -> p kp k n", kp=2, k=2)
                    t_aug = xd.tile(
                        [2, QB + KEYS], _BF16, tag="aug", name="aug"
                    )
                    nc.gpsimd.dma_start(t_aug[:], aug.ap())

            # PE warmup: HAM clock gate is cold (1.2 GHz) until ~3.4us of
            # sustained activity; bridge the wait for the first key chunk.
            warm = xd.tile([128, QB], _BF16, tag="warm", name="warm")
            nc.vector.memset(warm[:], 0.0)
            # ACT sqrt table preload (~2.7us) rides the startup phase
            nc.scalar.activation(
                warm[:, 0:1], warm[:, 0:1], sqrt, bias=0.0, scale=1.0
            )
            # 6 dummies ~2.6us (cold) end right as k0+q land, so the PE
            # busy-span runs unbroken into the real matmuls: the free-
            # running HAM window needs ~2 windows (~7us) of gapless
            # activity before it reliably ungates to 2.4 GHz.
            wps = pp.tile([128, NS * 512], _F32, tag="ps", name="wps")
            for _ in range(6):
                nc.tensor.matmul(
                    wps[:, 0:QB], warm[:, 0:128], warm[:], start=True, stop=True
                )

            for c in range(NC):
                psg = pp.tile([128, NS * 512], _F32, tag="ps", name=f"ps{c}")
                o = op.tile([128, NS * 512], _BF16, tag="o", name=f"o{c}")
                # kp-major: consecutive matmuls hit different PSUM banks
                # so fills pipeline (same-bank back-to-back accumulation
                # exposes the ~250-cycle drain). Augs pair s0,s1 then
                # s2,s3 so ACT+store can run per supertile half.
                for kp in (0, 1):
                    for s in range(NS):
                        nc.tensor.matmul(
                            psg[:, s * 512 : (s + 1) * 512],
                            qv[:, kp, :, s * 128 : (s + 1) * 128],
                            kv[c][:, kp, :, :],
                            start=(kp == 0),
                            stop=False,
                            perf_mode=dr,
                        )
                for s in range(NS):
                    nc.tensor.matmul(
                        psg[:, s * 512 : (s + 1) * 512],
                        t_aug[:, s * 128 : (s + 1) * 128],
                        t_aug[:, QB + c * 512 : QB + (c + 1) * 512],
                        start=False,
                        stop=True,
                    )
                for h in (0, 1):
                    hl = slice(h * 1024, (h + 1) * 1024)
                    nc.scalar.activation(
                        o[:, hl], psg[:, hl], sqrt, bias=0.0, scale=1.0
                    )
                    dst = out.ap()[
                        :, c * 2048 + h * 1024 : c * 2048 + (h + 1) * 1024
                    ]
                    eng = nc.gpsimd if h == 0 else nc.sync
                    eng.dma_start(dst, o[:, hl])

    nc.compile()
    _nc_cache["nc"] = nc
    return nc


def _ring(c):
    return [(c + t) % NCORES for t in range(RB)]


def _prep_inputs(x: np.ndarray):
    x = np.ascontiguousarray(x, dtype=np.float32)
    x8 = x.astype(ml_dtypes.float8_e4m3)       # keys [N, D]
    q8 = (-2.0 * x).astype(ml_dtypes.float8_e4m3)
    sqv = np.einsum("nd,nd->n", x.astype(np.float64), x.astype(np.float64))
    sqb = sqv.astype(ml_dtypes.bfloat16)
    ones = np.ones(N, dtype=ml_dtypes.bfloat16)

    in_maps = []
    for c in range(NCORES):
        r0 = c * QB
        keycols = np.concatenate(
            [np.arange(r * QB, (r + 1) * QB) for r in _ring(c)]
        )
        # keys: [p, chunk, ksub, n] with feature 128*ksub+p of key keycols[.]
        kc = x8[keycols, :].reshape(NC, 512, 4, 128)  # [c, n, k, p]
        xp_pack = kc.transpose(3, 0, 2, 1).reshape(128, NC * 4 * 512)
        # queries: [p, ksub, j]
        qc = q8[r0 : r0 + QB, :].reshape(QB, 4, 128)
        q_pack = qc.transpose(2, 1, 0).reshape(128, 4 * QB)
        aug_pack = np.empty((2, QB + KEYS), dtype=ml_dtypes.bfloat16)
        aug_pack[0, 0:QB] = sqb[r0 : r0 + QB]
        aug_pack[1, 0:QB] = ones[0:QB]
        aug_pack[0, QB:] = ones[0:KEYS]
        aug_pack[1, QB:] = sqb[keycols]
        in_maps.append(
            {
                "xp": np.ascontiguousarray(xp_pack),
                "q": np.ascontiguousarray(q_pack),
                "aug": np.ascontiguousarray(aug_pack),
            }
        )
    return in_maps


def run(x: np.ndarray, trace: bool = False, tmpdir: str | None = None):
    nc = _build()
    in_maps = _prep_inputs(x)
    res = run_bass_kernel_spmd(
        nc, in_maps, list(range(NCORES)), trace=trace, tmpdir=tmpdir
    )
    full = np.empty((N, N), dtype=np.float32)
    for c in range(NCORES):
        o = res.results[c]["out"].astype(np.float32)
        # [p, c, s, n] -> blk[q = s*128+p, key = c*512+n]
        blk = o.reshape(128, NC, NS, 512).transpose(2, 0, 1, 3).reshape(QB, KEYS)
        for t, r in enumerate(_ring(c)):
            b = blk[:, t * QB : (t + 1) * QB]  # [queries blk c, keys blk r]
            full[r * QB : (r + 1) * QB, c * QB : (c + 1) * QB] = b.T
            if t in (1, 2, 3):  # ring distance 1..3: mirror
                full[c * QB : (c + 1) * QB, r * QB : (r + 1) * QB] = b
    np.fill_diagonal(full, 0.0)
    return full, res


def kernel(x: np.ndarray) -> np.ndarray:
    out, _ = run(x, trace=False)
    return out
